# revision 28
# baseline (speedup 1.0000x reference)
"""Trainium2 Bass kernel for the MHA-with-diagonal-softmax module.

Computation (per batch b):
    q = rope(x @ Wq.T), k = rope(x @ Wk.T), v = x @ Wv.T      (per head, DH=128)
    sumexp[s,h] = sum_k exp(q_h[s] . k_h[k] * DH^-0.5)
    diag[s,h]   = q_h[s] . k_h[s] * DH^-0.5
    w = exp(diag) / sumexp
    out = (w * v) @ Wo.T

Sharding: 8 cores = 2 (batch) x 4 (head groups of 4 heads).
Each core computes q/k/v for its 4 heads in transposed [head_dim, seq]
layout, the per-position softmax-diagonal weights, and a partial output
projection (its heads' rows of Wo), written as 2 head-pair partials that
the host sums.

Performance structure: the kernel is one near-continuous PE matmul
stream.  K proj and Q proj (head 0) run first (input DMAs are chunked
seq-major so compute starts after ~2.5 MB instead of 10 MB).  All
remaining matmul work (Q proj heads 1-3, diag, V proj, output proj of
pair 0) lives in a filler queue that is drained between score blocks,
so the per-block exp() on the scalar engine never stalls the PE.  Score
PSUM is 2x[128,1024] double-buffered.  exp(diag) rows are precomputed
the moment each head's diag exists, shortening the pair transform
chains; a reserve of output-proj fillers covers the pair-1 transform
window before the tail.

On-chip dtype is fp16 (same PE throughput as bf16, 8x lower rounding
error - matters because exp() amplifies absolute score error), with fp32
PSUM accumulation everywhere.
"""

import numpy as np
from contextlib import ExitStack
from collections import deque

# Problem constants (hardcoded per harness contract).
B, S, D, H, DH = 2, 2048, 2048, 16, 128
HPC = 4            # heads per core
NHL = HPC * DH     # 512 local head dims per core
KB = D // 128      # 16 contraction blocks
SB = S // 128      # 16 seq blocks of 128
SC = S // 512      # 4 seq/emb chunks of 512
NCORES = 8

_CACHE = {}


def _build_nc():
    import concourse.bass as bass
    import concourse.tile as tile
    from concourse import bacc, mybir
    from concourse.masks import make_identity

    F16 = mybir.dt.float16
    F32 = mybir.dt.float32
    AF = mybir.ActivationFunctionType
    AX = mybir.AxisListType

    # Bacc (not raw Bass): its compile() splits multi-sem waits into
    # event-semaphore instructions - HW allows at most 1 wait per inst.
    nc = bacc.Bacc("TRN2", target_bir_lowering=False, debug=False)

    xT = nc.dram_tensor("xT", [D, S], F16, kind="ExternalInput").ap()
    wq = nc.dram_tensor("wq", [D, NHL], F16, kind="ExternalInput").ap()
    wk = nc.dram_tensor("wk", [D, NHL], F16, kind="ExternalInput").ap()
    wv = nc.dram_tensor("wv", [D, NHL], F16, kind="ExternalInput").ap()
    wo = nc.dram_tensor("wo", [NHL, D], F16, kind="ExternalInput").ap()
    ropeA = nc.dram_tensor("ropeA", [128, S], F16, kind="ExternalInput").ap()
    ropeB = nc.dram_tensor("ropeB", [128, S], F16, kind="ExternalInput").ap()
    y = nc.dram_tensor("y", [2, S, D], F16, kind="ExternalOutput").ap()

    # [kb, sc, 128, 512] view of x for chunked loads
    xT_c = xT.rearrange("(a p) (c w) -> a c p w", p=128, w=512)
    wq_r = wq.rearrange("(a p) m -> a p m", p=128)
    wk_r = wk.rearrange("(a p) m -> a p m", p=128)
    wv_r = wv.rearrange("(a p) m -> a p m", p=128)
    wo_r = wo.rearrange("(h p) n -> h p n", p=128)

    with tile.TileContext(nc) as tc, ExitStack() as ctx:
        pool = ctx.enter_context(tc.tile_pool(name="sb", bufs=1))
        pp = ctx.enter_context(tc.tile_pool(name="ps", bufs=1, space="PSUM"))

        # ---- constants ----
        ra = pool.tile([128, S], F16, name="ra")
        rb = pool.tile([128, S], F16, name="rb")
        # SWDGE: a wide HWDGE DMA fans out over several HW queues, and a
        # DVE/ACT consumer then needs one sync-wait per queue, exceeding
        # the instruction's wait-slot budget at compile time.
        nc.gpsimd.dma_start(ra[:, :], ropeA[:, :])
        nc.gpsimd.dma_start(rb[:, :], ropeB[:, :])
        ident = pool.tile([128, 128], F32, name="ident")
        make_identity(nc, ident[:, :])
        ones1 = pool.tile([128, 128], F16, name="ones1")
        nc.gpsimd.memset(ones1[:, :], 1.0)

        # ---- weight + x loads, ordered by first use ----
        def load_w(src_r, nblk, tag="w"):
            t = pool.tile([128, nblk, 512 * (KB // nblk)], F16, name="wt",
                          tag=tag, bufs=2)
            for i in range(nblk):
                nc.sync.dma_start(t[:, i, :], src_r[i])
            return t

        xsb = pool.tile([128, KB, S], F16, name="xsb")
        wkt = pool.tile([128, KB, 512], F16, name="wkt", tag="w", bufs=2)

        def load_x_sc(sc):
            for kb in range(KB):
                nc.sync.dma_start(xsb[:, kb, sc * 512:(sc + 1) * 512],
                                  xT_c[kb, sc])

        # wk and the first x chunk interleaved kb-by-kb so the first
        # projection group can stream DMA-paced from ~1.5us; then the
        # rest of x sc-major (the order K proj consumes it), wq between.
        for kb in range(KB):
            nc.sync.dma_start(wkt[:, kb, :], wk_r[kb])
            nc.sync.dma_start(xsb[:, kb, 0:512], xT_c[kb, 0])
        load_x_sc(1)
        wqt = load_w(wq_r, KB)
        load_x_sc(2)
        load_x_sc(3)

        # ---- persistent q/k/v head tiles ([head_dim, seq] layout) ----
        qh = [pool.tile([128, S], F16, name=f"qh{h}") for h in range(HPC)]
        kh = [pool.tile([128, S], F16, name=f"kh{h}") for h in range(HPC)]
        vh = [pool.tile([128, S], F16, name=f"vh{h}") for h in range(HPC)]

        # per-head row vectors live at partition 32*h (engine ops only
        # support start partitions that are multiples of 32)
        ds_diag = pool.tile([128, S], F32, name="ds_diag")
        expd = pool.tile([128, S], F16, name="expd")
        ds_sum = pool.tile([128, S], F16, name="ds_sum")
        w4 = pool.tile([128, S], F16, name="w4")
        # col sq = sumexp over all keys for the positions of block sq
        sumf = [pool.tile([128, SB], F32, name=f"sumf{h}")
                for h in range(HPC)]

        # ================= building blocks =================
        def proj_mms(wt, mt, sc, ps, kbs):
            for kb in kbs:
                nc.tensor.matmul(
                    ps[:, :],
                    wt[:, kb, mt * 128:(mt + 1) * 128],
                    xsb[:, kb, sc * 512:(sc + 1) * 512],
                    start=(kb == 0), stop=(kb == KB - 1))

        def proj_chunk(wt, dests, mt, sc, evac="act"):
            # dests[mt][:, sc-chunk] <- (wt[:, :, mt] block).T @ x chunk
            ps = pp.tile([128, 512], F32, name="mmps", tag="mm", bufs=2)
            proj_mms(wt, mt, sc, ps, range(KB))
            dst = dests[mt][:, sc * 512:(sc + 1) * 512]
            if evac == "act":
                nc.scalar.activation(dst, ps[:, :], AF.Copy)
            else:
                nc.vector.tensor_copy(dst, ps[:, :])

        def proj_unit(wt, dests, mt, sc, evac="act"):
            # one chunk as an atomic filler unit.  Atomicity matters: the
            # 16-matmul PSUM accumulation group must not interleave with
            # another 'mm'-tag allocation (same-bank reuse would clear
            # has_written mid-group).
            return (3460, lambda: proj_chunk(wt, dests, mt, sc, evac))

        def rope(dst, chunks=(0, 1)):
            # dst (in place): top = te*cos - to*sin ; bottom = te*sin + to*cos
            # ra = [cosT; cosT], rb = [-sinT; sinT]; swap = halves exchanged.
            for c in chunks:
                sl = slice(c * 1024, (c + 1) * 1024)
                # SWDGE (gpsimd) keeps this 1 queue -> 1 sem; a wide HWDGE
                # sbuf->sbuf DMA fans out over many queues and blows the
                # consumer's sync-wait slot budget.
                swp = pool.tile([128, 1024], F16, name="swp", tag="swp", bufs=2)
                nc.gpsimd.dma_start(swp[0:64, :], dst[64:128, sl])
                nc.gpsimd.dma_start(swp[64:128, :], dst[0:64, sl])
                u = pool.tile([128, 1024], F16, name="u", tag="sc", bufs=2)
                nc.vector.tensor_mul(u[:, :], dst[:, sl], ra[:, sl])
                v2 = pool.tile([128, 1024], F16, name="v2", tag="sc", bufs=2)
                nc.vector.tensor_mul(v2[:, :], swp[:, :], rb[:, sl])
                nc.vector.tensor_add(dst[:, sl], u[:, :], v2[:, :])

        def diag_unit(h, c):
            # ds_diag[32h, c-half] = per-position q.k (fp16 products,
            # fp32 psum accumulation via ones-matmul column sum)
            hp = 32 * h
            sl = slice(c * 1024, (c + 1) * 1024)
            pr = pool.tile([128, 1024], F16, name="pr", tag="pr", bufs=2)
            nc.vector.tensor_mul(pr[:, :], qh[h][:, sl], kh[h][:, sl])
            for cc in range(2):
                dps = pp.tile([128, 512], F32, name="dps", tag="mm", bufs=2)
                nc.tensor.matmul(dps[:, :], ones1[:, :],
                                 pr[:, cc * 512:(cc + 1) * 512],
                                 start=True, stop=True)
                o = (2 * c + cc) * 512
                nc.scalar.activation(ds_diag[hp:hp + 1, o:o + 512],
                                     dps[hp:hp + 1, :], AF.Copy)

        def expd_row(h):
            hp = 32 * h
            nc.scalar.activation(expd[hp:hp + 1, :], ds_diag[hp:hp + 1, :],
                                 AF.Exp)

        # accumulate the two half-sums of each sq into adjacent columns;
        # head_sum_tail adds them (a DVE reduce of the exp output would
        # cost ~2.3us/sq - reductions don't get the 16-bit 2x rate)
        sumh = [pool.tile([128, SB, 2], F32, name=f"sumh{h}")
                for h in range(HPC)]

        def sco_block(h, sq, half):
            # one 128x1024 score tile -> exp with fused key-axis row sum
            sps = pp.tile([128, 1024], F32, name="sps", tag="sco", bufs=2)
            for cc in range(2):
                o = half * 1024 + cc * 512
                nc.tensor.matmul(sps[:, cc * 512:(cc + 1) * 512],
                                 qh[h][:, sq * 128:(sq + 1) * 128],
                                 kh[h][:, o:o + 512],
                                 start=True, stop=True)
            ex = pool.tile([128, 1024], F16, name="ex", tag="ex", bufs=2)
            nc.scalar.activation(ex[:, :], sps[:, :], AF.Exp,
                                 accum_out=sumh[h][:, sq, half:half + 1])

        rsh = [pool.tile([128, SB], F32, name=f"rsh{h}")
               for h in range(HPC)]

        def hst_part(h, c0, c1):
            # sumexp columns [c0:c1): halves-add -> recip -> transpose ->
            # ds_sum row segment -> w segment.  Split so most of the
            # serial chain runs mid-blocks (exps for sq<c1 are already
            # done) and only a 4-column stub remains at the head boundary.
            n = c1 - c0
            hp = 32 * h
            nc.vector.tensor_add(sumf[h][:, c0:c1], sumh[h][:, c0:c1, 0],
                                 sumh[h][:, c0:c1, 1])
            nc.vector.reciprocal(rsh[h][:, c0:c1], sumf[h][:, c0:c1])
            tps = pp.tile([128, 512], F32, name="tps", tag="mm", bufs=2)
            nc.tensor.transpose(tps[0:n, 0:128], rsh[h][:, c0:c1],
                                ident[:, :])
            st = pool.tile([16, 128], F16, name="st", tag="st", bufs=2)
            nc.vector.tensor_copy(st[0:n, :], tps[0:n, 0:128])
            # sync (HWDGE) queue: tiny transfer, no fan-out concern; the
            # gpsimd queue would head-of-line block this behind rope swap
            # DMAs that wait on late projection fillers
            nc.sync.dma_start(
                ds_sum[hp:hp + 1, c0 * 128:c1 * 128], st[0:n, :])
            nc.vector.tensor_mul(w4[hp:hp + 1, c0 * 128:c1 * 128],
                                 expd[hp:hp + 1, c0 * 128:c1 * 128],
                                 ds_sum[hp:hp + 1, c0 * 128:c1 * 128])

        def head_weights(h, fill):
            # w = exp(diag) * recip(sumexp); attn = w (bcast) * v, into kh.
            # expd row precomputed; broadcast matmul output is consumed
            # directly from PSUM by the DVE multiply (no staging copy).
            hp = 32 * h
            for ck in range(SC):
                bps = pp.tile([128, 512], F32, name="bps", tag="op", bufs=2)
                nc.tensor.matmul(bps[:, :], ones1[hp:hp + 1, :],
                                 w4[hp:hp + 1, ck * 512:(ck + 1) * 512],
                                 start=True, stop=True,
                                 tile_position=(hp, 0))
                sl = slice(ck * 512, (ck + 1) * 512)
                nc.vector.tensor_mul(kh[h][:, sl], bps[:, :], vh[h][:, sl])
                fill(220)

        op_count = [0]
        tail_mode = [False]
        ytb_box = [None]

        def oproj_group(p, sb, ncx, taper=False):
            # groups run ncx 0..3 for one sb-block; the four [128,512]
            # evacuations land in one staging tile and ship as a single
            # contiguous DMA (4KB rows, 1 sync-queue trigger per 4 groups).
            # taper: per-group DMAs for the last batches so the final
            # drain spreads across queues instead of one serial transfer.
            h0, h1 = 2 * p, 2 * p + 1
            wot = wot_box[0]
            op_count[0] += 1
            use_sco = tail_mode[0] and op_count[0] % 2
            if use_sco:
                # scores are done in the tail: borrow the sco banks for a
                # 4-deep psum rotation so the group stream is PE-paced
                big = pp.tile([128, 1024], F32, name="osps", tag="sco",
                              bufs=2)
                ps = big[:, 0:512]
            else:
                ps = pp.tile([128, 512], F32, name="ops", tag="op", bufs=2)
            for i, h in enumerate((h0, h1)):
                nc.tensor.matmul(
                    ps[:, :], kh[h][:, sb * 128:(sb + 1) * 128],
                    wot[:, h, ncx * 512:(ncx + 1) * 512],
                    start=(i == 0), stop=(i == 1))
            if taper:
                yt = pool.tile([128, 512], F16, name="ytt", tag="ytt",
                               bufs=4)
                dst = yt[:, :]
            else:
                if ncx == 0:
                    ytb_box[0] = pool.tile([128, S], F16, name="ytb",
                                           tag="yt", bufs=2)
                dst = ytb_box[0][:, ncx * 512:(ncx + 1) * 512]
            # DVE while ACT runs at the exp floor; alternate engines in
            # the PE-dense tail so neither one paces the group stream
            if use_sco:
                nc.scalar.activation(dst, ps[:, :], AF.Copy)
            else:
                nc.vector.tensor_copy(dst, ps[:, :])
            if taper:
                nc.sync.dma_start(
                    y[p, sb * 128:(sb + 1) * 128,
                      ncx * 512:(ncx + 1) * 512], dst)
            elif ncx == SC - 1:
                nc.sync.dma_start(
                    y[p, sb * 128:(sb + 1) * 128, :], ytb_box[0][:, :])

        # ================= filler queue =================
        # Units of (pe_cost_ns, emit_fn), drained between score blocks to
        # keep the PE streaming while ACT runs the exps.
        fillers = deque()
        fill_debt = [0.0]

        def fill(budget):
            fill_debt[0] += budget
            while fillers and fillers[0][0] <= fill_debt[0]:
                cost, fn = fillers.popleft()
                fn()
                fill_debt[0] -= cost

        def drain(dq):
            while dq:
                _, fn = dq.popleft()
                fn()

        # ================= emission =================
        # dense PE lead-in.  K proj sc0 runs as 4 parallel accumulation
        # groups (2 'sco' + 2 'mm' psum tiles) with matmuls kb-major, so
        # the PE streams in DMA arrival order from ~1.5us instead of
        # waiting for the full 4MB.  Ropes are emitted the moment their
        # tensor completes so the DVE overlaps the remaining PE chunks.
        lead_big = [pp.tile([128, 1024], F32, name="lps", tag="sco",
                            bufs=2) for _ in range(2)]
        lead_ps = [t[:, 0:512] for t in lead_big] + [
            pp.tile([128, 512], F32, name="mmps", tag="mm", bufs=2)
            for _ in range(2)]
        for kb in range(KB):
            for mt in range(HPC):
                nc.tensor.matmul(
                    lead_ps[mt], wkt[:, kb, mt * 128:(mt + 1) * 128],
                    xsb[:, kb, 0:512],
                    start=(kb == 0), stop=(kb == KB - 1))
        for mt in range(HPC):
            nc.scalar.activation(kh[mt][:, 0:512], lead_ps[mt], AF.Copy)
        for sc in range(1, SC - 1):
            for mt in range(HPC):
                proj_chunk(wkt, kh, mt, sc)
        for mt in range(HPC):
            proj_chunk(wkt, kh, mt, SC - 1)
            rope(kh[mt])
        # wv reuses wk's slot, wo reuses wq's slot (tag bufs=2); the loads
        # self-delay on the WAR semaphore of the previous consumer.
        wvt = load_w(wv_r, KB)
        proj_chunk(wqt, qh, 0, 0)
        proj_chunk(wqt, qh, 0, 1)
        rope(qh[0], (0,))
        proj_chunk(wqt, qh, 0, 2)
        proj_chunk(wqt, qh, 0, 3)
        rope(qh[0], (1,))

        # Emission-time progress flags for the force-drain guards below
        # (all bookkeeping is emission-time python, fully deterministic).
        q_ready = [True] + [False] * (HPC - 1)
        v_done = [0] * HPC
        e_done = [False] * HPC

        def mark(fn, after):
            def wrapped():
                fn()
                after()
            return wrapped

        # queue: diag/expd h0, Q proj h1 (+rope/diag/expd), V proj h0/h1,
        # Q proj h2/h3, V proj h2/h3; pair-0 output proj appended later.
        for c in range(2):
            fillers.append((440, lambda c=c: diag_unit(0, c)))
        fillers.append(
            (0, mark(lambda: expd_row(0),
                     lambda: e_done.__setitem__(0, True))))
        wot_box = []

        def queue_qhead(hq):
            for sc in range(SC):
                fillers.append(proj_unit(wqt, qh, hq, sc))
            fillers.append(
                (0, mark(lambda hq=hq: rope(qh[hq]),
                         lambda hq=hq: q_ready.__setitem__(hq, True))))
            for c in range(2):
                fillers.append((440, lambda hq=hq, c=c: diag_unit(hq, c)))
            fillers.append(
                (0, mark(lambda hq=hq: expd_row(hq),
                         lambda hq=hq: e_done.__setitem__(hq, True))))

        def queue_vhead(mt):
            for sc in range(SC):
                fillers.append(
                    (3460, mark(
                        lambda mt=mt, sc=sc:
                        proj_chunk(wvt, vh, mt, sc, evac="dve"),
                        lambda mt=mt:
                        v_done.__setitem__(mt, v_done[mt] + 1))))

        queue_qhead(1)
        queue_vhead(0)
        queue_vhead(1)
        queue_qhead(2)
        queue_qhead(3)
        # wo load directly after the last wqt-consuming unit
        fillers.append((0, lambda: wot_box.append(load_w(wo_r, HPC))))
        queue_vhead(2)
        queue_vhead(3)

        FILL_A = 780    # ns of filler per score block (ACT exp+accum
        FILL_B = 620    # paces a block at ~1.3us; 432ns are score MMs)

        reserve = deque()

        def rfill(budget):
            fill_debt[0] += budget
            while reserve and reserve[0][0] <= fill_debt[0]:
                cost, fn = reserve.popleft()
                fn()
                fill_debt[0] -= cost

        def force(cond):
            # pop fillers (in order) until an emission-order precondition
            # holds; keeps DVE/PE FIFO deps acyclic regardless of budgets
            while fillers and not cond():
                _, fn = fillers.popleft()
                fn()
            assert cond()

        def emit_head_blocks(h, per_block):
            force(lambda: q_ready[h])
            for sq in range(SB):
                for half in range(2):
                    sco_block(h, sq, half)
                    fill(per_block)
                if sq == 12:
                    force(lambda: e_done[h])
                    hst_part(h, 0, 12)

        def pair_tail(h, f):
            # per-head: finish the last sumexp columns, then w + attn,
            # right after the head's last score block; ck3 goes last so
            # its chain hides under the ck0-2 broadcast+attn work
            f(4000)
            hst_part(h, 12, SB)
            force(lambda: v_done[h] >= SC)
            f(2500)
            head_weights(h, f)

        emit_head_blocks(0, FILL_A)
        pair_tail(0, fill)
        emit_head_blocks(1, FILL_A)
        pair_tail(1, fill)

        # pair-0 output projection becomes available (24 units reserved
        # as PE cover for the head-3 transform window)
        oq = [(s_, n_) for s_ in range(SB) for n_ in range(SC)]
        for i, (s_, n_) in enumerate(oq):
            unit = (432, lambda s_=s_, n_=n_: oproj_group(0, s_, n_))
            (reserve if i >= len(oq) - 24 else fillers).append(unit)

        emit_head_blocks(2, FILL_B)
        pair_tail(2, fill)
        emit_head_blocks(3, FILL_B)
        drain(fillers)
        pair_tail(3, rfill)
        tail_mode[0] = True
        drain(reserve)

        # ---- tail: pair 1 output projection ----
        for sb in range(SB):
            for ncx in range(SC):
                oproj_group(1, sb, ncx, taper=(sb >= SB - 2))

    nc.compile()
    return nc


def _get_nc():
    if "nc" not in _CACHE:
        _CACHE["nc"] = _build_nc()
    return _CACHE["nc"]


_PERM = np.concatenate([np.arange(0, DH, 2), np.arange(1, DH, 2)])


def _host_inputs(x, rope_cos, rope_sin, Wq, Wk, Wv, Wo):
    """Build the 8 per-core input maps."""
    f16 = np.float16
    cosT = np.ascontiguousarray(np.asarray(rope_cos, np.float32)[0, :, 0, :].T)
    sinT = np.ascontiguousarray(np.asarray(rope_sin, np.float32)[0, :, 0, :].T)
    ra = np.concatenate([cosT, cosT], 0).astype(f16)
    rb = np.concatenate([-sinT, sinT], 0).astype(f16)

    Wq = np.asarray(Wq, np.float32)
    Wk = np.asarray(Wk, np.float32)
    Wv = np.asarray(Wv, np.float32)
    Wo = np.asarray(Wo, np.float32)
    x = np.asarray(x, np.float32)

    xTb = [np.ascontiguousarray(x[b].T).astype(f16) for b in range(B)]
    scale = DH ** -0.5

    in_maps = []
    for core in range(NCORES):
        b, g = divmod(core, HPC)
        hs = g * HPC
        rows = np.concatenate(
            [h * DH + _PERM for h in range(hs, hs + HPC)])      # deinterleave
        rows_v = np.arange(hs * DH, (hs + HPC) * DH)
        in_maps.append({
            "xT": xTb[b],
            "wq": np.ascontiguousarray((Wq[rows] * scale).T).astype(f16),
            "wk": np.ascontiguousarray(Wk[rows].T).astype(f16),
            "wv": np.ascontiguousarray(Wv[rows_v].T).astype(f16),
            "wo": np.ascontiguousarray(Wo[:, rows_v].T).astype(f16),
            "ropeA": ra,
            "ropeB": rb,
        })
    return in_maps


def kernel(x, rope_cos, rope_sin, Wq, Wk, Wv, Wo, _trace=False, _trace_cores=None):
    from concourse.bass_utils import run_bass_kernel_spmd

    nc = _get_nc()
    in_maps = _host_inputs(x, rope_cos, rope_sin, Wq, Wk, Wv, Wo)
    res = run_bass_kernel_spmd(nc, in_maps, list(range(NCORES)),
                               trace=_trace, trace_cores=_trace_cores)
    _CACHE["last_result"] = res

    out = np.zeros((B, S, D), np.float32)
    for core in range(NCORES):
        b = core // HPC
        out[b] += res.results[core]["y"].astype(np.float32).sum(axis=0)
    return out


# revision 29
# speedup vs baseline: 1.0136x; 1.0136x over previous
"""Trainium2 Bass kernel for the MHA-with-diagonal-softmax module.

Computation (per batch b):
    q = rope(x @ Wq.T), k = rope(x @ Wk.T), v = x @ Wv.T      (per head, DH=128)
    sumexp[s,h] = sum_k exp(q_h[s] . k_h[k] * DH^-0.5)
    diag[s,h]   = q_h[s] . k_h[s] * DH^-0.5
    w = exp(diag) / sumexp
    out = (w * v) @ Wo.T

Sharding: 8 cores = 2 (batch) x 4 (head groups of 4 heads).
Each core computes q/k/v for its 4 heads in transposed [head_dim, seq]
layout, the per-position softmax-diagonal weights, and a partial output
projection (its heads' rows of Wo), written as 2 head-pair partials that
the host sums.

Performance structure: the kernel is one near-continuous PE matmul
stream.  K proj and Q proj (head 0) run first (input DMAs are chunked
seq-major so compute starts after ~2.5 MB instead of 10 MB).  All
remaining matmul work (Q proj heads 1-3, diag, V proj, output proj of
pair 0) lives in a filler queue that is drained between score blocks,
so the per-block exp() on the scalar engine never stalls the PE.  Score
PSUM is 2x[128,1024] double-buffered.  exp(diag) rows are precomputed
the moment each head's diag exists, shortening the pair transform
chains; a reserve of output-proj fillers covers the pair-1 transform
window before the tail.

On-chip dtype is fp16 (same PE throughput as bf16, 8x lower rounding
error - matters because exp() amplifies absolute score error), with fp32
PSUM accumulation everywhere.
"""

import numpy as np
from contextlib import ExitStack
from collections import deque

# Problem constants (hardcoded per harness contract).
B, S, D, H, DH = 2, 2048, 2048, 16, 128
HPC = 4            # heads per core
NHL = HPC * DH     # 512 local head dims per core
KB = D // 128      # 16 contraction blocks
SB = S // 128      # 16 seq blocks of 128
SC = S // 512      # 4 seq/emb chunks of 512
NCORES = 8

_CACHE = {}


def _build_nc():
    import concourse.bass as bass
    import concourse.tile as tile
    from concourse import bacc, mybir
    from concourse.masks import make_identity

    F16 = mybir.dt.float16
    F32 = mybir.dt.float32
    AF = mybir.ActivationFunctionType
    AX = mybir.AxisListType

    # Bacc (not raw Bass): its compile() splits multi-sem waits into
    # event-semaphore instructions - HW allows at most 1 wait per inst.
    nc = bacc.Bacc("TRN2", target_bir_lowering=False, debug=False)

    xT = nc.dram_tensor("xT", [D, S], F16, kind="ExternalInput").ap()
    wq = nc.dram_tensor("wq", [D, NHL], F16, kind="ExternalInput").ap()
    wk = nc.dram_tensor("wk", [D, NHL], F16, kind="ExternalInput").ap()
    wv = nc.dram_tensor("wv", [D, NHL], F16, kind="ExternalInput").ap()
    wo = nc.dram_tensor("wo", [NHL, D], F16, kind="ExternalInput").ap()
    ropeA = nc.dram_tensor("ropeA", [128, S], F16, kind="ExternalInput").ap()
    ropeB = nc.dram_tensor("ropeB", [128, S], F16, kind="ExternalInput").ap()
    y = nc.dram_tensor("y", [2, S, D], F16, kind="ExternalOutput").ap()

    # [kb, sc, 128, 512] view of x for chunked loads
    xT_c = xT.rearrange("(a p) (c w) -> a c p w", p=128, w=512)
    wq_r = wq.rearrange("(a p) m -> a p m", p=128)
    wk_r = wk.rearrange("(a p) m -> a p m", p=128)
    wv_r = wv.rearrange("(a p) m -> a p m", p=128)
    wo_r = wo.rearrange("(h p) n -> h p n", p=128)

    with tile.TileContext(nc) as tc, ExitStack() as ctx:
        pool = ctx.enter_context(tc.tile_pool(name="sb", bufs=1))
        pp = ctx.enter_context(tc.tile_pool(name="ps", bufs=1, space="PSUM"))

        # ---- constants ----
        ra = pool.tile([128, S], F16, name="ra")
        rb = pool.tile([128, S], F16, name="rb")
        # SWDGE: a wide HWDGE DMA fans out over several HW queues, and a
        # DVE/ACT consumer then needs one sync-wait per queue, exceeding
        # the instruction's wait-slot budget at compile time.
        nc.gpsimd.dma_start(ra[:, :], ropeA[:, :])
        nc.gpsimd.dma_start(rb[:, :], ropeB[:, :])
        ident = pool.tile([128, 128], F32, name="ident")
        make_identity(nc, ident[:, :])
        ones1 = pool.tile([128, 128], F16, name="ones1")
        nc.gpsimd.memset(ones1[:, :], 1.0)

        # ---- weight + x loads, ordered by first use ----
        def load_w(src_r, nblk, tag="w"):
            t = pool.tile([128, nblk, 512 * (KB // nblk)], F16, name="wt",
                          tag=tag, bufs=2)
            for i in range(nblk):
                nc.sync.dma_start(t[:, i, :], src_r[i])
            return t

        xsb = pool.tile([128, KB, S], F16, name="xsb")
        wkt = pool.tile([128, KB, 512], F16, name="wkt", tag="w", bufs=2)

        def load_x_sc(sc):
            for kb in range(KB):
                nc.sync.dma_start(xsb[:, kb, sc * 512:(sc + 1) * 512],
                                  xT_c[kb, sc])

        # wk and the first x chunk interleaved kb-by-kb so the first
        # projection group can stream DMA-paced from ~1.5us; then the
        # rest of x sc-major (the order K proj consumes it), wq between.
        for kb in range(KB):
            nc.sync.dma_start(wkt[:, kb, :], wk_r[kb])
            nc.sync.dma_start(xsb[:, kb, 0:512], xT_c[kb, 0])
        load_x_sc(1)
        wqt = load_w(wq_r, KB)
        load_x_sc(2)
        load_x_sc(3)

        # ---- persistent q/k/v head tiles ([head_dim, seq] layout) ----
        qh = [pool.tile([128, S], F16, name=f"qh{h}") for h in range(HPC)]
        kh = [pool.tile([128, S], F16, name=f"kh{h}") for h in range(HPC)]
        vh = [pool.tile([128, S], F16, name=f"vh{h}") for h in range(HPC)]

        # per-head row vectors live at partition 32*h (engine ops only
        # support start partitions that are multiples of 32)
        ds_diag = pool.tile([128, S], F32, name="ds_diag")
        expd = pool.tile([128, S], F16, name="expd")
        ds_sum = pool.tile([128, S], F16, name="ds_sum")
        w4 = pool.tile([128, S], F16, name="w4")
        # col sq = sumexp over all keys for the positions of block sq
        sumf = [pool.tile([128, SB], F32, name=f"sumf{h}")
                for h in range(HPC)]

        # ================= building blocks =================
        def proj_mms(wt, mt, sc, ps, kbs):
            for kb in kbs:
                nc.tensor.matmul(
                    ps[:, :],
                    wt[:, kb, mt * 128:(mt + 1) * 128],
                    xsb[:, kb, sc * 512:(sc + 1) * 512],
                    start=(kb == 0), stop=(kb == KB - 1))

        def proj_chunk(wt, dests, mt, sc, evac="act"):
            # dests[mt][:, sc-chunk] <- (wt[:, :, mt] block).T @ x chunk
            ps = pp.tile([128, 512], F32, name="mmps", tag="mm", bufs=2)
            proj_mms(wt, mt, sc, ps, range(KB))
            dst = dests[mt][:, sc * 512:(sc + 1) * 512]
            if evac == "act":
                nc.scalar.activation(dst, ps[:, :], AF.Copy)
            else:
                nc.vector.tensor_copy(dst, ps[:, :])

        def proj_unit(wt, dests, mt, sc, evac="act"):
            # one chunk as an atomic filler unit.  Atomicity matters: the
            # 16-matmul PSUM accumulation group must not interleave with
            # another 'mm'-tag allocation (same-bank reuse would clear
            # has_written mid-group).
            return (3460, lambda: proj_chunk(wt, dests, mt, sc, evac))

        def rope(dst, chunks=(0, 1)):
            # dst (in place): top = te*cos - to*sin ; bottom = te*sin + to*cos
            # ra = [cosT; cosT], rb = [-sinT; sinT]; swap = halves exchanged.
            for c in chunks:
                sl = slice(c * 1024, (c + 1) * 1024)
                # SWDGE (gpsimd) keeps this 1 queue -> 1 sem; a wide HWDGE
                # sbuf->sbuf DMA fans out over many queues and blows the
                # consumer's sync-wait slot budget.
                swp = pool.tile([128, 1024], F16, name="swp", tag="swp", bufs=2)
                nc.gpsimd.dma_start(swp[0:64, :], dst[64:128, sl])
                nc.gpsimd.dma_start(swp[64:128, :], dst[0:64, sl])
                u = pool.tile([128, 1024], F16, name="u", tag="sc", bufs=2)
                nc.vector.tensor_mul(u[:, :], dst[:, sl], ra[:, sl])
                v2 = pool.tile([128, 1024], F16, name="v2", tag="sc", bufs=2)
                nc.vector.tensor_mul(v2[:, :], swp[:, :], rb[:, sl])
                nc.vector.tensor_add(dst[:, sl], u[:, :], v2[:, :])

        def diag_unit(h, c):
            # ds_diag[32h, c-half] = per-position q.k (fp16 products,
            # fp32 psum accumulation via ones-matmul column sum)
            hp = 32 * h
            sl = slice(c * 1024, (c + 1) * 1024)
            pr = pool.tile([128, 1024], F16, name="pr", tag="pr", bufs=2)
            nc.vector.tensor_mul(pr[:, :], qh[h][:, sl], kh[h][:, sl])
            for cc in range(2):
                dps = pp.tile([128, 512], F32, name="dps", tag="mm", bufs=2)
                nc.tensor.matmul(dps[:, :], ones1[:, :],
                                 pr[:, cc * 512:(cc + 1) * 512],
                                 start=True, stop=True)
                o = (2 * c + cc) * 512
                nc.scalar.activation(ds_diag[hp:hp + 1, o:o + 512],
                                     dps[hp:hp + 1, :], AF.Copy)

        def expd_row(h):
            hp = 32 * h
            nc.scalar.activation(expd[hp:hp + 1, :], ds_diag[hp:hp + 1, :],
                                 AF.Exp)

        # accumulate the two half-sums of each sq into adjacent columns;
        # head_sum_tail adds them (a DVE reduce of the exp output would
        # cost ~2.3us/sq - reductions don't get the 16-bit 2x rate)
        sumh = [pool.tile([128, SB, 2], F32, name=f"sumh{h}")
                for h in range(HPC)]

        def sco_block(h, sq, half):
            # one 128x1024 score tile -> exp with fused key-axis row sum
            sps = pp.tile([128, 1024], F32, name="sps", tag="sco", bufs=2)
            for cc in range(2):
                o = half * 1024 + cc * 512
                nc.tensor.matmul(sps[:, cc * 512:(cc + 1) * 512],
                                 qh[h][:, sq * 128:(sq + 1) * 128],
                                 kh[h][:, o:o + 512],
                                 start=True, stop=True)
            ex = pool.tile([128, 1024], F16, name="ex", tag="ex", bufs=2)
            nc.scalar.activation(ex[:, :], sps[:, :], AF.Exp,
                                 accum_out=sumh[h][:, sq, half:half + 1])

        rsh = [pool.tile([128, SB], F32, name=f"rsh{h}")
               for h in range(HPC)]

        def hst_part(h, c0, c1):
            # sumexp columns [c0:c1): halves-add -> recip -> transpose ->
            # ds_sum row segment -> w segment.  Split so most of the
            # serial chain runs mid-blocks (exps for sq<c1 are already
            # done) and only a 4-column stub remains at the head boundary.
            n = c1 - c0
            hp = 32 * h
            nc.vector.tensor_add(sumf[h][:, c0:c1], sumh[h][:, c0:c1, 0],
                                 sumh[h][:, c0:c1, 1])
            nc.vector.reciprocal(rsh[h][:, c0:c1], sumf[h][:, c0:c1])
            tps = pp.tile([128, 512], F32, name="tps", tag="mm", bufs=2)
            nc.tensor.transpose(tps[0:n, 0:128], rsh[h][:, c0:c1],
                                ident[:, :])
            st = pool.tile([16, 128], F16, name="st", tag="st", bufs=2)
            nc.vector.tensor_copy(st[0:n, :], tps[0:n, 0:128])
            # sync (HWDGE) queue: tiny transfer, no fan-out concern; the
            # gpsimd queue would head-of-line block this behind rope swap
            # DMAs that wait on late projection fillers
            nc.sync.dma_start(
                ds_sum[hp:hp + 1, c0 * 128:c1 * 128], st[0:n, :])
            nc.vector.tensor_mul(w4[hp:hp + 1, c0 * 128:c1 * 128],
                                 expd[hp:hp + 1, c0 * 128:c1 * 128],
                                 ds_sum[hp:hp + 1, c0 * 128:c1 * 128])

        def head_weights(h, fill):
            # w = exp(diag) * recip(sumexp); attn = w (bcast) * v, into kh.
            # expd row precomputed; broadcast matmul output is consumed
            # directly from PSUM by the DVE multiply (no staging copy).
            hp = 32 * h
            for ck in range(SC):
                bps = pp.tile([128, 512], F32, name="bps", tag="op", bufs=2)
                nc.tensor.matmul(bps[:, :], ones1[hp:hp + 1, :],
                                 w4[hp:hp + 1, ck * 512:(ck + 1) * 512],
                                 start=True, stop=True,
                                 tile_position=(hp, 0))
                sl = slice(ck * 512, (ck + 1) * 512)
                nc.vector.tensor_mul(kh[h][:, sl], bps[:, :], vh[h][:, sl])
                fill(220)

        op_count = [0]
        tail_mode = [False]
        ytb_box = [None]

        # output pairs: the host sums all partials, so pair membership is
        # free.  (1,2) completes after head 2 - its output projection
        # feeds head 3's score blocks; (0,3) fills the tail.
        PAIRS = [(1, 2), (0, 3)]

        def oproj_group(p, sb, ncx, taper=False):
            # groups run ncx 0..3 for one sb-block; the four [128,512]
            # evacuations land in one staging tile and ship as a single
            # contiguous DMA (4KB rows, 1 sync-queue trigger per 4 groups).
            # taper: per-group DMAs for the last batches so the final
            # drain spreads across queues instead of one serial transfer.
            h0, h1 = PAIRS[p]
            wot = wot_box[0]
            op_count[0] += 1
            use_sco = tail_mode[0] and op_count[0] % 2
            if use_sco:
                # scores are done in the tail: borrow the sco banks for a
                # 4-deep psum rotation so the group stream is PE-paced
                big = pp.tile([128, 1024], F32, name="osps", tag="sco",
                              bufs=2)
                ps = big[:, 0:512]
            else:
                ps = pp.tile([128, 512], F32, name="ops", tag="op", bufs=2)
            for i, h in enumerate((h0, h1)):
                nc.tensor.matmul(
                    ps[:, :], kh[h][:, sb * 128:(sb + 1) * 128],
                    wot[:, h, ncx * 512:(ncx + 1) * 512],
                    start=(i == 0), stop=(i == 1))
            if taper:
                yt = pool.tile([128, 512], F16, name="ytt", tag="ytt",
                               bufs=4)
                dst = yt[:, :]
            else:
                if ncx == 0:
                    ytb_box[0] = pool.tile([128, S], F16, name="ytb",
                                           tag="yt", bufs=2)
                dst = ytb_box[0][:, ncx * 512:(ncx + 1) * 512]
            # DVE while ACT runs at the exp floor; alternate engines in
            # the PE-dense tail so neither one paces the group stream
            if use_sco:
                nc.scalar.activation(dst, ps[:, :], AF.Copy)
            else:
                nc.vector.tensor_copy(dst, ps[:, :])
            if taper:
                nc.sync.dma_start(
                    y[p, sb * 128:(sb + 1) * 128,
                      ncx * 512:(ncx + 1) * 512], dst)
            elif ncx == SC - 1:
                nc.sync.dma_start(
                    y[p, sb * 128:(sb + 1) * 128, :], ytb_box[0][:, :])

        # ================= filler queue =================
        # Units of (pe_cost_ns, emit_fn), drained between score blocks to
        # keep the PE streaming while ACT runs the exps.
        fillers = deque()
        fill_debt = [0.0]

        def fill(budget):
            fill_debt[0] += budget
            while fillers and fillers[0][0] <= fill_debt[0]:
                cost, fn = fillers.popleft()
                fn()
                fill_debt[0] -= cost

        def drain(dq):
            while dq:
                _, fn = dq.popleft()
                fn()

        # ================= emission =================
        # dense PE lead-in.  K proj sc0 runs as 4 parallel accumulation
        # groups (2 'sco' + 2 'mm' psum tiles) with matmuls kb-major, so
        # the PE streams in DMA arrival order from ~1.5us instead of
        # waiting for the full 4MB.  Ropes are emitted the moment their
        # tensor completes so the DVE overlaps the remaining PE chunks.
        lead_big = [pp.tile([128, 1024], F32, name="lps", tag="sco",
                            bufs=2) for _ in range(2)]
        lead_ps = [t[:, 0:512] for t in lead_big] + [
            pp.tile([128, 512], F32, name="mmps", tag="mm", bufs=2)
            for _ in range(2)]
        for kb in range(KB):
            for mt in range(HPC):
                nc.tensor.matmul(
                    lead_ps[mt], wkt[:, kb, mt * 128:(mt + 1) * 128],
                    xsb[:, kb, 0:512],
                    start=(kb == 0), stop=(kb == KB - 1))
        for mt in range(HPC):
            nc.scalar.activation(kh[mt][:, 0:512], lead_ps[mt], AF.Copy)
        for sc in range(1, SC - 1):
            for mt in range(HPC):
                proj_chunk(wkt, kh, mt, sc)
        for mt in range(HPC):
            proj_chunk(wkt, kh, mt, SC - 1)
            rope(kh[mt])
        # wv reuses wk's slot, wo reuses wq's slot (tag bufs=2); the loads
        # self-delay on the WAR semaphore of the previous consumer.
        wvt = load_w(wv_r, KB)
        proj_chunk(wqt, qh, 0, 0)
        proj_chunk(wqt, qh, 0, 1)
        rope(qh[0], (0,))
        proj_chunk(wqt, qh, 0, 2)
        proj_chunk(wqt, qh, 0, 3)
        rope(qh[0], (1,))

        # Emission-time progress flags for the force-drain guards below
        # (all bookkeeping is emission-time python, fully deterministic).
        q_ready = [True] + [False] * (HPC - 1)
        v_done = [0] * HPC
        e_done = [False] * HPC

        def mark(fn, after):
            def wrapped():
                fn()
                after()
            return wrapped

        # queue: diag/expd h0, Q proj h1 (+rope/diag/expd), V proj h0/h1,
        # Q proj h2/h3, V proj h2/h3; pair-0 output proj appended later.
        for c in range(2):
            fillers.append((440, lambda c=c: diag_unit(0, c)))
        fillers.append(
            (0, mark(lambda: expd_row(0),
                     lambda: e_done.__setitem__(0, True))))
        wot_box = []

        def queue_qhead(hq):
            for sc in range(SC):
                fillers.append(proj_unit(wqt, qh, hq, sc))
            fillers.append(
                (0, mark(lambda hq=hq: rope(qh[hq]),
                         lambda hq=hq: q_ready.__setitem__(hq, True))))
            for c in range(2):
                fillers.append((440, lambda hq=hq, c=c: diag_unit(hq, c)))
            fillers.append(
                (0, mark(lambda hq=hq: expd_row(hq),
                         lambda hq=hq: e_done.__setitem__(hq, True))))

        def queue_vhead(mt):
            for sc in range(SC):
                fillers.append(
                    (3460, mark(
                        lambda mt=mt, sc=sc:
                        proj_chunk(wvt, vh, mt, sc, evac="dve"),
                        lambda mt=mt:
                        v_done.__setitem__(mt, v_done[mt] + 1))))

        queue_qhead(1)
        queue_vhead(0)
        queue_vhead(1)
        queue_qhead(2)
        queue_vhead(2)
        queue_qhead(3)
        # wo load directly after the last wqt-consuming unit
        fillers.append((0, lambda: wot_box.append(load_w(wo_r, HPC))))
        queue_vhead(3)

        FILL_A = 840    # ns of filler per score block: ACT exp+accum
        FILL_B = 840    # paces a block at ~1.3us, 432ns are score MMs;
        # PE period must stay >= the ACT period or the PE micro-waits
        # on score PSUM reuse every block and HAM drops to half clock

        reserve = deque()

        def rfill(budget):
            fill_debt[0] += budget
            while reserve and reserve[0][0] <= fill_debt[0]:
                cost, fn = reserve.popleft()
                fn()
                fill_debt[0] -= cost

        def force(cond):
            # pop fillers (in order) until an emission-order precondition
            # holds; keeps DVE/PE FIFO deps acyclic regardless of budgets
            while fillers and not cond():
                _, fn = fillers.popleft()
                fn()
            assert cond()

        def emit_head_blocks(h, per_block):
            force(lambda: q_ready[h])
            for sq in range(SB):
                for half in range(2):
                    sco_block(h, sq, half)
                    fill(per_block)
                if sq == 12:
                    force(lambda: e_done[h])
                    hst_part(h, 0, 12)

        def pair_tail(h, f):
            # per-head: finish the last sumexp columns, then w + attn,
            # right after the head's last score block; ck3 goes last so
            # its chain hides under the ck0-2 broadcast+attn work
            f(4000)
            hst_part(h, 12, SB)
            force(lambda: v_done[h] >= SC)
            f(2500)
            head_weights(h, f)

        emit_head_blocks(0, FILL_A)
        pair_tail(0, fill)
        emit_head_blocks(1, FILL_A)
        pair_tail(1, fill)
        emit_head_blocks(2, FILL_B)
        pair_tail(2, fill)

        # pair (1,2) output projection becomes available; it feeds head
        # 3's blocks (8 units reserved for the head-3 transform window)
        oq = [(s_, n_) for s_ in range(SB) for n_ in range(SC)]
        for i, (s_, n_) in enumerate(oq):
            unit = (432, lambda s_=s_, n_=n_: oproj_group(0, s_, n_))
            (reserve if i >= len(oq) - 8 else fillers).append(unit)

        emit_head_blocks(3, FILL_B)
        drain(fillers)
        pair_tail(3, rfill)
        tail_mode[0] = True
        drain(reserve)

        # ---- tail: pair (0,3) output projection ----
        for sb in range(SB):
            for ncx in range(SC):
                oproj_group(1, sb, ncx, taper=(sb >= SB - 2))

    nc.compile()
    return nc


def _get_nc():
    if "nc" not in _CACHE:
        _CACHE["nc"] = _build_nc()
    return _CACHE["nc"]


_PERM = np.concatenate([np.arange(0, DH, 2), np.arange(1, DH, 2)])


def _host_inputs(x, rope_cos, rope_sin, Wq, Wk, Wv, Wo):
    """Build the 8 per-core input maps."""
    f16 = np.float16
    cosT = np.ascontiguousarray(np.asarray(rope_cos, np.float32)[0, :, 0, :].T)
    sinT = np.ascontiguousarray(np.asarray(rope_sin, np.float32)[0, :, 0, :].T)
    ra = np.concatenate([cosT, cosT], 0).astype(f16)
    rb = np.concatenate([-sinT, sinT], 0).astype(f16)

    Wq = np.asarray(Wq, np.float32)
    Wk = np.asarray(Wk, np.float32)
    Wv = np.asarray(Wv, np.float32)
    Wo = np.asarray(Wo, np.float32)
    x = np.asarray(x, np.float32)

    xTb = [np.ascontiguousarray(x[b].T).astype(f16) for b in range(B)]
    scale = DH ** -0.5

    in_maps = []
    for core in range(NCORES):
        b, g = divmod(core, HPC)
        hs = g * HPC
        rows = np.concatenate(
            [h * DH + _PERM for h in range(hs, hs + HPC)])      # deinterleave
        rows_v = np.arange(hs * DH, (hs + HPC) * DH)
        in_maps.append({
            "xT": xTb[b],
            "wq": np.ascontiguousarray((Wq[rows] * scale).T).astype(f16),
            "wk": np.ascontiguousarray(Wk[rows].T).astype(f16),
            "wv": np.ascontiguousarray(Wv[rows_v].T).astype(f16),
            "wo": np.ascontiguousarray(Wo[:, rows_v].T).astype(f16),
            "ropeA": ra,
            "ropeB": rb,
        })
    return in_maps


def kernel(x, rope_cos, rope_sin, Wq, Wk, Wv, Wo, _trace=False, _trace_cores=None):
    from concourse.bass_utils import run_bass_kernel_spmd

    nc = _get_nc()
    in_maps = _host_inputs(x, rope_cos, rope_sin, Wq, Wk, Wv, Wo)
    res = run_bass_kernel_spmd(nc, in_maps, list(range(NCORES)),
                               trace=_trace, trace_cores=_trace_cores)
    _CACHE["last_result"] = res

    out = np.zeros((B, S, D), np.float32)
    for core in range(NCORES):
        b = core // HPC
        out[b] += res.results[core]["y"].astype(np.float32).sum(axis=0)
    return out


# revision 30
# speedup vs baseline: 1.0187x; 1.0050x over previous
"""Trainium2 Bass kernel for the MHA-with-diagonal-softmax module.

Computation (per batch b):
    q = rope(x @ Wq.T), k = rope(x @ Wk.T), v = x @ Wv.T      (per head, DH=128)
    sumexp[s,h] = sum_k exp(q_h[s] . k_h[k] * DH^-0.5)
    diag[s,h]   = q_h[s] . k_h[s] * DH^-0.5
    w = exp(diag) / sumexp
    out = (w * v) @ Wo.T

Sharding: 8 cores = 2 (batch) x 4 (head groups of 4 heads).
Each core computes q/k/v for its 4 heads in transposed [head_dim, seq]
layout, the per-position softmax-diagonal weights, and a partial output
projection (its heads' rows of Wo), written as 2 head-pair partials that
the host sums.

Performance structure: the kernel is one near-continuous PE matmul
stream.  K proj and Q proj (head 0) run first (input DMAs are chunked
seq-major so compute starts after ~2.5 MB instead of 10 MB).  All
remaining matmul work (Q proj heads 1-3, diag, V proj, output proj of
pair 0) lives in a filler queue that is drained between score blocks,
so the per-block exp() on the scalar engine never stalls the PE.  Score
PSUM is 2x[128,1024] double-buffered.  exp(diag) rows are precomputed
the moment each head's diag exists, shortening the pair transform
chains; a reserve of output-proj fillers covers the pair-1 transform
window before the tail.

On-chip dtype is fp16 (same PE throughput as bf16, 8x lower rounding
error - matters because exp() amplifies absolute score error), with fp32
PSUM accumulation everywhere.
"""

import numpy as np
from contextlib import ExitStack
from collections import deque

# Problem constants (hardcoded per harness contract).
B, S, D, H, DH = 2, 2048, 2048, 16, 128
HPC = 4            # heads per core
NHL = HPC * DH     # 512 local head dims per core
KB = D // 128      # 16 contraction blocks
SB = S // 128      # 16 seq blocks of 128
SC = S // 512      # 4 seq/emb chunks of 512
NCORES = 8

_CACHE = {}


def _build_nc():
    import concourse.bass as bass
    import concourse.tile as tile
    from concourse import bacc, mybir
    from concourse.masks import make_identity

    F16 = mybir.dt.float16
    F32 = mybir.dt.float32
    AF = mybir.ActivationFunctionType
    AX = mybir.AxisListType

    # Bacc (not raw Bass): its compile() splits multi-sem waits into
    # event-semaphore instructions - HW allows at most 1 wait per inst.
    nc = bacc.Bacc("TRN2", target_bir_lowering=False, debug=False)

    xT = nc.dram_tensor("xT", [D, S], F16, kind="ExternalInput").ap()
    wq = nc.dram_tensor("wq", [D, NHL], F16, kind="ExternalInput").ap()
    wk = nc.dram_tensor("wk", [D, NHL], F16, kind="ExternalInput").ap()
    wv = nc.dram_tensor("wv", [D, NHL], F16, kind="ExternalInput").ap()
    wo = nc.dram_tensor("wo", [NHL, D], F16, kind="ExternalInput").ap()
    ropeA = nc.dram_tensor("ropeA", [128, S], F16, kind="ExternalInput").ap()
    ropeB = nc.dram_tensor("ropeB", [128, S], F16, kind="ExternalInput").ap()
    y = nc.dram_tensor("y", [2, S, D], F16, kind="ExternalOutput").ap()

    # [kb, sc, 128, 512] view of x for chunked loads
    xT_c = xT.rearrange("(a p) (c w) -> a c p w", p=128, w=512)
    wq_r = wq.rearrange("(a p) m -> a p m", p=128)
    wk_r = wk.rearrange("(a p) m -> a p m", p=128)
    wv_r = wv.rearrange("(a p) m -> a p m", p=128)
    wo_r = wo.rearrange("(h p) n -> h p n", p=128)

    with tile.TileContext(nc) as tc, ExitStack() as ctx:
        pool = ctx.enter_context(tc.tile_pool(name="sb", bufs=1))
        pp = ctx.enter_context(tc.tile_pool(name="ps", bufs=1, space="PSUM"))

        # ---- constants ----
        ra = pool.tile([128, S], F16, name="ra")
        rb = pool.tile([128, S], F16, name="rb")
        # SWDGE: a wide HWDGE DMA fans out over several HW queues, and a
        # DVE/ACT consumer then needs one sync-wait per queue, exceeding
        # the instruction's wait-slot budget at compile time.
        nc.gpsimd.dma_start(ra[:, :], ropeA[:, :])
        nc.gpsimd.dma_start(rb[:, :], ropeB[:, :])
        ident = pool.tile([128, 128], F32, name="ident")
        make_identity(nc, ident[:, :])
        ones1 = pool.tile([128, 128], F16, name="ones1")
        nc.gpsimd.memset(ones1[:, :], 1.0)

        # ---- weight + x loads, ordered by first use ----
        def load_w(src_r, nblk, tag="w"):
            t = pool.tile([128, nblk, 512 * (KB // nblk)], F16, name="wt",
                          tag=tag, bufs=2)
            for i in range(nblk):
                nc.sync.dma_start(t[:, i, :], src_r[i])
            return t

        xsb = pool.tile([128, KB, S], F16, name="xsb")
        wkt = pool.tile([128, KB, 512], F16, name="wkt", tag="w", bufs=2)

        def load_x_sc(sc):
            for kb in range(KB):
                nc.sync.dma_start(xsb[:, kb, sc * 512:(sc + 1) * 512],
                                  xT_c[kb, sc])

        # wk and the first x chunk interleaved kb-by-kb so the first
        # projection group can stream DMA-paced from ~1.5us; then the
        # rest of x sc-major (the order K proj consumes it), wq between.
        for kb in range(KB):
            nc.sync.dma_start(wkt[:, kb, :], wk_r[kb])
            nc.sync.dma_start(xsb[:, kb, 0:512], xT_c[kb, 0])
        load_x_sc(1)
        wqt = load_w(wq_r, KB)
        load_x_sc(2)
        load_x_sc(3)

        # ---- persistent q/k/v head tiles ([head_dim, seq] layout) ----
        qh = [pool.tile([128, S], F16, name=f"qh{h}") for h in range(HPC)]
        kh = [pool.tile([128, S], F16, name=f"kh{h}") for h in range(HPC)]
        vh = [pool.tile([128, S], F16, name=f"vh{h}") for h in range(HPC)]

        # per-head row vectors live at partition 32*h (engine ops only
        # support start partitions that are multiples of 32)
        ds_diag = pool.tile([128, S], F32, name="ds_diag")
        expd = pool.tile([128, S], F16, name="expd")
        ds_sum = pool.tile([128, S], F16, name="ds_sum")
        w4 = pool.tile([128, S], F16, name="w4")
        # col sq = sumexp over all keys for the positions of block sq
        sumf = [pool.tile([128, SB], F32, name=f"sumf{h}")
                for h in range(HPC)]

        # ================= building blocks =================
        def proj_mms(wt, mt, sc, ps, kbs):
            for kb in kbs:
                nc.tensor.matmul(
                    ps[:, :],
                    wt[:, kb, mt * 128:(mt + 1) * 128],
                    xsb[:, kb, sc * 512:(sc + 1) * 512],
                    start=(kb == 0), stop=(kb == KB - 1))

        def proj_chunk(wt, dests, mt, sc, evac="act"):
            # dests[mt][:, sc-chunk] <- (wt[:, :, mt] block).T @ x chunk
            ps = pp.tile([128, 512], F32, name="mmps", tag="mm", bufs=2)
            proj_mms(wt, mt, sc, ps, range(KB))
            dst = dests[mt][:, sc * 512:(sc + 1) * 512]
            if evac == "act":
                nc.scalar.activation(dst, ps[:, :], AF.Copy)
            else:
                nc.vector.tensor_copy(dst, ps[:, :])

        def proj_unit(wt, dests, mt, sc, evac="act"):
            # one chunk as an atomic filler unit.  Atomicity matters: the
            # 16-matmul PSUM accumulation group must not interleave with
            # another 'mm'-tag allocation (same-bank reuse would clear
            # has_written mid-group).
            return (3460, lambda: proj_chunk(wt, dests, mt, sc, evac))

        def rope(dst, chunks=(0, 1)):
            # dst (in place): top = te*cos - to*sin ; bottom = te*sin + to*cos
            # ra = [cosT; cosT], rb = [-sinT; sinT]; swap = halves exchanged.
            for c in chunks:
                sl = slice(c * 1024, (c + 1) * 1024)
                # SWDGE (gpsimd) keeps this 1 queue -> 1 sem; a wide HWDGE
                # sbuf->sbuf DMA fans out over many queues and blows the
                # consumer's sync-wait slot budget.
                swp = pool.tile([128, 1024], F16, name="swp", tag="swp", bufs=2)
                nc.gpsimd.dma_start(swp[0:64, :], dst[64:128, sl])
                nc.gpsimd.dma_start(swp[64:128, :], dst[0:64, sl])
                u = pool.tile([128, 1024], F16, name="u", tag="sc", bufs=2)
                nc.vector.tensor_mul(u[:, :], dst[:, sl], ra[:, sl])
                v2 = pool.tile([128, 1024], F16, name="v2", tag="sc", bufs=2)
                nc.vector.tensor_mul(v2[:, :], swp[:, :], rb[:, sl])
                nc.vector.tensor_add(dst[:, sl], u[:, :], v2[:, :])

        def diag_unit(h, c):
            # ds_diag[32h, c-half] = per-position q.k (fp16 products,
            # fp32 psum accumulation via ones-matmul column sum)
            hp = 32 * h
            sl = slice(c * 1024, (c + 1) * 1024)
            pr = pool.tile([128, 1024], F16, name="pr", tag="pr", bufs=2)
            nc.vector.tensor_mul(pr[:, :], qh[h][:, sl], kh[h][:, sl])
            for cc in range(2):
                dps = pp.tile([128, 512], F32, name="dps", tag="mm", bufs=2)
                nc.tensor.matmul(dps[:, :], ones1[:, :],
                                 pr[:, cc * 512:(cc + 1) * 512],
                                 start=True, stop=True)
                o = (2 * c + cc) * 512
                nc.vector.tensor_copy(ds_diag[hp:hp + 1, o:o + 512],
                                      dps[hp:hp + 1, :])

        def expd_row(h):
            hp = 32 * h
            nc.scalar.activation(expd[hp:hp + 1, :], ds_diag[hp:hp + 1, :],
                                 AF.Exp)

        # accumulate the two half-sums of each sq into adjacent columns;
        # head_sum_tail adds them (a DVE reduce of the exp output would
        # cost ~2.3us/sq - reductions don't get the 16-bit 2x rate)
        sumh = [pool.tile([128, SB, 2], F32, name=f"sumh{h}")
                for h in range(HPC)]

        def sco_block(h, sq, half):
            # one 128x1024 score tile -> exp with fused key-axis row sum
            sps = pp.tile([128, 1024], F32, name="sps", tag="sco", bufs=2)
            for cc in range(2):
                o = half * 1024 + cc * 512
                nc.tensor.matmul(sps[:, cc * 512:(cc + 1) * 512],
                                 qh[h][:, sq * 128:(sq + 1) * 128],
                                 kh[h][:, o:o + 512],
                                 start=True, stop=True)
            ex = pool.tile([128, 1024], F16, name="ex", tag="ex", bufs=2)
            nc.scalar.activation(ex[:, :], sps[:, :], AF.Exp,
                                 accum_out=sumh[h][:, sq, half:half + 1])

        rsh = [pool.tile([128, SB], F32, name=f"rsh{h}")
               for h in range(HPC)]

        def hst_part(h, c0, c1):
            # sumexp columns [c0:c1): halves-add -> recip -> transpose ->
            # ds_sum row segment -> w segment.  Split so most of the
            # serial chain runs mid-blocks (exps for sq<c1 are already
            # done) and only a 4-column stub remains at the head boundary.
            n = c1 - c0
            hp = 32 * h
            nc.vector.tensor_add(sumf[h][:, c0:c1], sumh[h][:, c0:c1, 0],
                                 sumh[h][:, c0:c1, 1])
            nc.vector.reciprocal(rsh[h][:, c0:c1], sumf[h][:, c0:c1])
            tps = pp.tile([128, 512], F32, name="tps", tag="mm", bufs=2)
            nc.tensor.transpose(tps[0:n, 0:128], rsh[h][:, c0:c1],
                                ident[:, :])
            st = pool.tile([16, 128], F16, name="st", tag="st", bufs=2)
            nc.vector.tensor_copy(st[0:n, :], tps[0:n, 0:128])
            # sync (HWDGE) queue: tiny transfer, no fan-out concern; the
            # gpsimd queue would head-of-line block this behind rope swap
            # DMAs that wait on late projection fillers
            nc.sync.dma_start(
                ds_sum[hp:hp + 1, c0 * 128:c1 * 128], st[0:n, :])
            nc.vector.tensor_mul(w4[hp:hp + 1, c0 * 128:c1 * 128],
                                 expd[hp:hp + 1, c0 * 128:c1 * 128],
                                 ds_sum[hp:hp + 1, c0 * 128:c1 * 128])

        def head_weights(h, fill):
            # w = exp(diag) * recip(sumexp); attn = w (bcast) * v, into kh.
            # expd row precomputed; broadcast matmul output is consumed
            # directly from PSUM by the DVE multiply (no staging copy).
            hp = 32 * h
            for ck in range(SC):
                bps = pp.tile([128, 512], F32, name="bps", tag="op", bufs=2)
                nc.tensor.matmul(bps[:, :], ones1[hp:hp + 1, :],
                                 w4[hp:hp + 1, ck * 512:(ck + 1) * 512],
                                 start=True, stop=True,
                                 tile_position=(hp, 0))
                sl = slice(ck * 512, (ck + 1) * 512)
                nc.vector.tensor_mul(kh[h][:, sl], bps[:, :], vh[h][:, sl])
                fill(220)

        op_count = [0]
        tail_mode = [False]
        ytb_box = [None]

        # output pairs: the host sums all partials, so pair membership is
        # free.  (1,2) completes after head 2 - its output projection
        # feeds head 3's score blocks; (0,3) fills the tail.
        PAIRS = [(1, 2), (0, 3)]

        def oproj_group(p, sb, ncx, taper=False):
            # groups run ncx 0..3 for one sb-block; the four [128,512]
            # evacuations land in one staging tile and ship as a single
            # contiguous DMA (4KB rows, 1 sync-queue trigger per 4 groups).
            # taper: per-group DMAs for the last batches so the final
            # drain spreads across queues instead of one serial transfer.
            h0, h1 = PAIRS[p]
            wot = wot_box[0]
            op_count[0] += 1
            use_sco = tail_mode[0] and op_count[0] % 2
            if use_sco:
                # scores are done in the tail: borrow the sco banks for a
                # 4-deep psum rotation so the group stream is PE-paced
                big = pp.tile([128, 1024], F32, name="osps", tag="sco",
                              bufs=2)
                ps = big[:, 0:512]
            else:
                ps = pp.tile([128, 512], F32, name="ops", tag="op", bufs=2)
            for i, h in enumerate((h0, h1)):
                nc.tensor.matmul(
                    ps[:, :], kh[h][:, sb * 128:(sb + 1) * 128],
                    wot[:, h, ncx * 512:(ncx + 1) * 512],
                    start=(i == 0), stop=(i == 1))
            if taper:
                yt = pool.tile([128, 512], F16, name="ytt", tag="ytt",
                               bufs=4)
                dst = yt[:, :]
            else:
                if ncx == 0:
                    ytb_box[0] = pool.tile([128, S], F16, name="ytb",
                                           tag="yt", bufs=2)
                dst = ytb_box[0][:, ncx * 512:(ncx + 1) * 512]
            # DVE while ACT runs at the exp floor; alternate engines in
            # the PE-dense tail so neither one paces the group stream
            if use_sco:
                nc.scalar.activation(dst, ps[:, :], AF.Copy)
            else:
                nc.vector.tensor_copy(dst, ps[:, :])
            if taper:
                nc.sync.dma_start(
                    y[p, sb * 128:(sb + 1) * 128,
                      ncx * 512:(ncx + 1) * 512], dst)
            elif ncx == SC - 1:
                nc.sync.dma_start(
                    y[p, sb * 128:(sb + 1) * 128, :], ytb_box[0][:, :])

        # ================= filler queue =================
        # Units of (pe_cost_ns, emit_fn), drained between score blocks to
        # keep the PE streaming while ACT runs the exps.
        fillers = deque()
        fill_debt = [0.0]

        def fill(budget):
            fill_debt[0] += budget
            while fillers and fillers[0][0] <= fill_debt[0]:
                cost, fn = fillers.popleft()
                fn()
                fill_debt[0] -= cost

        def drain(dq):
            while dq:
                _, fn = dq.popleft()
                fn()

        # ================= emission =================
        # dense PE lead-in.  K proj sc0 runs as 4 parallel accumulation
        # groups (2 'sco' + 2 'mm' psum tiles) with matmuls kb-major, so
        # the PE streams in DMA arrival order from ~1.5us instead of
        # waiting for the full 4MB.  Ropes are emitted the moment their
        # tensor completes so the DVE overlaps the remaining PE chunks.
        lead_big = [pp.tile([128, 1024], F32, name="lps", tag="sco",
                            bufs=2) for _ in range(2)]
        lead_ps = [t[:, 0:512] for t in lead_big] + [
            pp.tile([128, 512], F32, name="mmps", tag="mm", bufs=2)
            for _ in range(2)]
        for kb in range(KB):
            for mt in range(HPC):
                nc.tensor.matmul(
                    lead_ps[mt], wkt[:, kb, mt * 128:(mt + 1) * 128],
                    xsb[:, kb, 0:512],
                    start=(kb == 0), stop=(kb == KB - 1))
        for mt in range(HPC):
            nc.scalar.activation(kh[mt][:, 0:512], lead_ps[mt], AF.Copy)
        for sc in range(1, SC - 1):
            for mt in range(HPC):
                proj_chunk(wkt, kh, mt, sc)
        for mt in range(HPC):
            proj_chunk(wkt, kh, mt, SC - 1)
            rope(kh[mt])
        # wv reuses wk's slot, wo reuses wq's slot (tag bufs=2); the loads
        # self-delay on the WAR semaphore of the previous consumer.
        wvt = load_w(wv_r, KB)
        proj_chunk(wqt, qh, 0, 0)
        proj_chunk(wqt, qh, 0, 1)
        rope(qh[0], (0,))
        proj_chunk(wqt, qh, 0, 2)
        proj_chunk(wqt, qh, 0, 3)
        rope(qh[0], (1,))

        # Emission-time progress flags for the force-drain guards below
        # (all bookkeeping is emission-time python, fully deterministic).
        q_ready = [True] + [False] * (HPC - 1)
        v_done = [0] * HPC
        e_done = [False] * HPC

        def mark(fn, after):
            def wrapped():
                fn()
                after()
            return wrapped

        # queue: diag/expd h0, Q proj h1 (+rope/diag/expd), V proj h0/h1,
        # Q proj h2/h3, V proj h2/h3; pair-0 output proj appended later.
        for c in range(2):
            fillers.append((440, lambda c=c: diag_unit(0, c)))
        fillers.append(
            (0, mark(lambda: expd_row(0),
                     lambda: e_done.__setitem__(0, True))))
        wot_box = []

        def queue_qhead(hq):
            for sc in range(SC):
                fillers.append(proj_unit(wqt, qh, hq, sc, evac="dve"))
            fillers.append((0, lambda hq=hq: rope(qh[hq], (0,))))
            fillers.append(
                (0, mark(lambda hq=hq: rope(qh[hq], (1,)),
                         lambda hq=hq: q_ready.__setitem__(hq, True))))
            for c in range(2):
                fillers.append((440, lambda hq=hq, c=c: diag_unit(hq, c)))
            fillers.append(
                (0, mark(lambda hq=hq: expd_row(hq),
                         lambda hq=hq: e_done.__setitem__(hq, True))))

        def queue_vhead(mt):
            for sc in range(SC):
                fillers.append(
                    (3460, mark(
                        lambda mt=mt, sc=sc:
                        proj_chunk(wvt, vh, mt, sc, evac="dve"),
                        lambda mt=mt:
                        v_done.__setitem__(mt, v_done[mt] + 1))))

        queue_qhead(1)
        queue_vhead(0)
        queue_vhead(1)
        queue_qhead(2)
        queue_vhead(2)
        queue_qhead(3)
        # wo load directly after the last wqt-consuming unit
        fillers.append((0, lambda: wot_box.append(load_w(wo_r, HPC))))
        queue_vhead(3)

        FILL_A = 840    # ns of filler per score block: ACT exp+accum
        FILL_B = 840    # paces a block at ~1.3us, 432ns are score MMs;
        # PE period must stay >= the ACT period or the PE micro-waits
        # on score PSUM reuse every block and HAM drops to half clock

        reserve = deque()

        def rfill(budget):
            fill_debt[0] += budget
            while reserve and reserve[0][0] <= fill_debt[0]:
                cost, fn = reserve.popleft()
                fn()
                fill_debt[0] -= cost

        def force(cond):
            # pop fillers (in order) until an emission-order precondition
            # holds; keeps DVE/PE FIFO deps acyclic regardless of budgets
            while fillers and not cond():
                _, fn = fillers.popleft()
                fn()
            assert cond()

        def emit_head_blocks(h, per_block):
            force(lambda: q_ready[h])
            for sq in range(SB):
                for half in range(2):
                    sco_block(h, sq, half)
                    fill(per_block)
                if sq == 12:
                    force(lambda: e_done[h])
                    hst_part(h, 0, 12)

        def pair_tail(h, f):
            # per-head: finish the last sumexp columns, then w + attn,
            # right after the head's last score block; ck3 goes last so
            # its chain hides under the ck0-2 broadcast+attn work
            f(4000)
            hst_part(h, 12, SB)
            force(lambda: v_done[h] >= SC)
            f(2500)
            head_weights(h, f)

        emit_head_blocks(0, FILL_A)
        pair_tail(0, fill)
        emit_head_blocks(1, FILL_A)
        pair_tail(1, fill)
        emit_head_blocks(2, FILL_B)
        pair_tail(2, fill)

        # pair (1,2) output projection becomes available; it feeds head
        # 3's blocks (8 units reserved for the head-3 transform window)
        oq = [(s_, n_) for s_ in range(SB) for n_ in range(SC)]
        for i, (s_, n_) in enumerate(oq):
            unit = (432, lambda s_=s_, n_=n_: oproj_group(0, s_, n_))
            (reserve if i >= len(oq) - 8 else fillers).append(unit)

        emit_head_blocks(3, FILL_B)
        drain(fillers)
        pair_tail(3, rfill)
        tail_mode[0] = True
        drain(reserve)

        # ---- tail: pair (0,3) output projection ----
        for sb in range(SB):
            for ncx in range(SC):
                oproj_group(1, sb, ncx, taper=(sb >= SB - 2))

    nc.compile()
    return nc


def _get_nc():
    if "nc" not in _CACHE:
        _CACHE["nc"] = _build_nc()
    return _CACHE["nc"]


_PERM = np.concatenate([np.arange(0, DH, 2), np.arange(1, DH, 2)])


def _host_inputs(x, rope_cos, rope_sin, Wq, Wk, Wv, Wo):
    """Build the 8 per-core input maps."""
    f16 = np.float16
    cosT = np.ascontiguousarray(np.asarray(rope_cos, np.float32)[0, :, 0, :].T)
    sinT = np.ascontiguousarray(np.asarray(rope_sin, np.float32)[0, :, 0, :].T)
    ra = np.concatenate([cosT, cosT], 0).astype(f16)
    rb = np.concatenate([-sinT, sinT], 0).astype(f16)

    Wq = np.asarray(Wq, np.float32)
    Wk = np.asarray(Wk, np.float32)
    Wv = np.asarray(Wv, np.float32)
    Wo = np.asarray(Wo, np.float32)
    x = np.asarray(x, np.float32)

    xTb = [np.ascontiguousarray(x[b].T).astype(f16) for b in range(B)]
    scale = DH ** -0.5

    in_maps = []
    for core in range(NCORES):
        b, g = divmod(core, HPC)
        hs = g * HPC
        rows = np.concatenate(
            [h * DH + _PERM for h in range(hs, hs + HPC)])      # deinterleave
        rows_v = np.arange(hs * DH, (hs + HPC) * DH)
        in_maps.append({
            "xT": xTb[b],
            "wq": np.ascontiguousarray((Wq[rows] * scale).T).astype(f16),
            "wk": np.ascontiguousarray(Wk[rows].T).astype(f16),
            "wv": np.ascontiguousarray(Wv[rows_v].T).astype(f16),
            "wo": np.ascontiguousarray(Wo[:, rows_v].T).astype(f16),
            "ropeA": ra,
            "ropeB": rb,
        })
    return in_maps


def kernel(x, rope_cos, rope_sin, Wq, Wk, Wv, Wo, _trace=False, _trace_cores=None):
    from concourse.bass_utils import run_bass_kernel_spmd

    nc = _get_nc()
    in_maps = _host_inputs(x, rope_cos, rope_sin, Wq, Wk, Wv, Wo)
    res = run_bass_kernel_spmd(nc, in_maps, list(range(NCORES)),
                               trace=_trace, trace_cores=_trace_cores)
    _CACHE["last_result"] = res

    out = np.zeros((B, S, D), np.float32)
    for core in range(NCORES):
        b = core // HPC
        out[b] += res.results[core]["y"].astype(np.float32).sum(axis=0)
    return out


# revision 31
# speedup vs baseline: 1.1105x; 1.0901x over previous
"""Trainium2 Bass kernel for the MHA-with-diagonal-softmax module.

Computation (per batch b):
    q = rope(x @ Wq.T), k = rope(x @ Wk.T), v = x @ Wv.T      (per head, DH=128)
    sumexp[s,h] = sum_k exp(q_h[s] . k_h[k] * DH^-0.5)
    diag[s,h]   = q_h[s] . k_h[s] * DH^-0.5
    w = exp(diag) / sumexp
    out = (w * v) @ Wo.T

Sharding: 8 cores = 2 (batch) x 4 (head groups of 4 heads).
Each core computes q/k/v for its 4 heads in transposed [head_dim, seq]
layout, the per-position softmax-diagonal weights, and a partial output
projection (its heads' rows of Wo), written as 2 head-pair partials that
the host sums.

Performance structure: the kernel is one near-continuous PE matmul
stream.  K proj and Q proj (head 0) run first (input DMAs are chunked
seq-major so compute starts after ~2.5 MB instead of 10 MB).  All
remaining matmul work (Q proj heads 1-3, diag, V proj, output proj of
pair 0) lives in a filler queue that is drained between score blocks,
so the per-block exp() on the scalar engine never stalls the PE.  Score
PSUM is 2x[128,1024] double-buffered.  exp(diag) rows are precomputed
the moment each head's diag exists, shortening the pair transform
chains; a reserve of output-proj fillers covers the pair-1 transform
window before the tail.

On-chip dtype is fp16 (same PE throughput as bf16, 8x lower rounding
error - matters because exp() amplifies absolute score error), with fp32
PSUM accumulation everywhere.
"""

import numpy as np
from contextlib import ExitStack
from collections import deque

# Problem constants (hardcoded per harness contract).
B, S, D, H, DH = 2, 2048, 2048, 16, 128
HPC = 4            # heads per core
NHL = HPC * DH     # 512 local head dims per core
KB = D // 128      # 16 contraction blocks
SB = S // 128      # 16 seq blocks of 128
SC = S // 512      # 4 seq/emb chunks of 512
NCORES = 8

_CACHE = {}


def _build_nc():
    import concourse.bass as bass
    import concourse.tile as tile
    from concourse import bacc, mybir
    from concourse.masks import make_identity

    F16 = mybir.dt.float16
    F32 = mybir.dt.float32
    AF = mybir.ActivationFunctionType
    AX = mybir.AxisListType

    # Bacc (not raw Bass): its compile() splits multi-sem waits into
    # event-semaphore instructions - HW allows at most 1 wait per inst.
    nc = bacc.Bacc("TRN2", target_bir_lowering=False, debug=False)

    xT = nc.dram_tensor("xT", [D, S], F16, kind="ExternalInput").ap()
    wq = nc.dram_tensor("wq", [D, NHL], F16, kind="ExternalInput").ap()
    wk = nc.dram_tensor("wk", [D, NHL], F16, kind="ExternalInput").ap()
    wv = nc.dram_tensor("wv", [D, NHL], F16, kind="ExternalInput").ap()
    wo = nc.dram_tensor("wo", [NHL, D], F16, kind="ExternalInput").ap()
    ropeA = nc.dram_tensor("ropeA", [128, S], F16, kind="ExternalInput").ap()
    ropeB = nc.dram_tensor("ropeB", [128, S], F16, kind="ExternalInput").ap()
    y = nc.dram_tensor("y", [2, S, D], F16, kind="ExternalOutput").ap()

    xT_r = xT.rearrange("(a p) s -> a p s", p=128)
    wq_r = wq.rearrange("(a p) m -> a p m", p=128)
    wk_r = wk.rearrange("(a p) m -> a p m", p=128)
    wv_r = wv.rearrange("(a p) m -> a p m", p=128)
    wo_r = wo.rearrange("(h p) n -> h p n", p=128)

    with tile.TileContext(nc) as tc, ExitStack() as ctx:
        pool = ctx.enter_context(tc.tile_pool(name="sb", bufs=1))
        pp = ctx.enter_context(tc.tile_pool(name="ps", bufs=1, space="PSUM"))

        # ---- constants ----
        ra = pool.tile([128, S], F16, name="ra")
        rb = pool.tile([128, S], F16, name="rb")
        # SWDGE: a wide HWDGE DMA fans out over several HW queues, and a
        # DVE/ACT consumer then needs one sync-wait per queue, exceeding
        # the instruction's wait-slot budget at compile time.
        nc.gpsimd.dma_start(ra[:, :], ropeA[:, :])
        nc.gpsimd.dma_start(rb[:, :], ropeB[:, :])
        ident = pool.tile([128, 128], F32, name="ident")
        make_identity(nc, ident[:, :])
        ones1 = pool.tile([128, 128], F16, name="ones1")
        nc.gpsimd.memset(ones1[:, :], 1.0)

        # ---- weight + x loads, ordered by first use ----
        def load_w(src_r, nblk, tag="w"):
            t = pool.tile([128, nblk, 512 * (KB // nblk)], F16, name="wt",
                          tag=tag, bufs=2)
            for i in range(nblk):
                nc.sync.dma_start(t[:, i, :], src_r[i])
            return t

        xsb = pool.tile([128, KB, S], F16, name="xsb")
        wkt = pool.tile([128, KB, 512], F16, name="wkt", tag="w", bufs=2)

        # wk and x interleaved kb-by-kb: full [128,2048] x rows (4KB DMA
        # rows, near-peak bandwidth) consumed kb-major by the first
        # 8-group projection phase below
        for kb in range(KB):
            nc.sync.dma_start(wkt[:, kb, :], wk_r[kb])
            nc.sync.dma_start(xsb[:, kb, :], xT_r[kb])
        wqt = load_w(wq_r, KB)

        # ---- persistent q/k/v head tiles ([head_dim, seq] layout) ----
        qh = [pool.tile([128, S], F16, name=f"qh{h}") for h in range(HPC)]
        kh = [pool.tile([128, S], F16, name=f"kh{h}") for h in range(HPC)]
        vh = [pool.tile([128, S], F16, name=f"vh{h}") for h in range(HPC)]

        # per-head row vectors live at partition 32*h (engine ops only
        # support start partitions that are multiples of 32)
        ds_diag = pool.tile([128, S], F32, name="ds_diag")
        expd = pool.tile([128, S], F16, name="expd")
        ds_sum = pool.tile([128, S], F16, name="ds_sum")
        w4 = pool.tile([128, S], F16, name="w4")
        # col sq = sumexp over all keys for the positions of block sq
        sumf = [pool.tile([128, SB], F32, name=f"sumf{h}")
                for h in range(HPC)]

        # ================= building blocks =================
        def proj_mms(wt, mt, sc, ps, kbs):
            for kb in kbs:
                nc.tensor.matmul(
                    ps[:, :],
                    wt[:, kb, mt * 128:(mt + 1) * 128],
                    xsb[:, kb, sc * 512:(sc + 1) * 512],
                    start=(kb == 0), stop=(kb == KB - 1))

        def proj_chunk(wt, dests, mt, sc, evac="act"):
            # dests[mt][:, sc-chunk] <- (wt[:, :, mt] block).T @ x chunk
            ps = pp.tile([128, 512], F32, name="mmps", tag="mm", bufs=2)
            proj_mms(wt, mt, sc, ps, range(KB))
            dst = dests[mt][:, sc * 512:(sc + 1) * 512]
            if evac == "act":
                nc.scalar.activation(dst, ps[:, :], AF.Copy)
            else:
                nc.vector.tensor_copy(dst, ps[:, :])

        def proj_unit(wt, dests, mt, sc, evac="act"):
            # one chunk as an atomic filler unit.  Atomicity matters: the
            # 16-matmul PSUM accumulation group must not interleave with
            # another 'mm'-tag allocation (same-bank reuse would clear
            # has_written mid-group).
            return (3460, lambda: proj_chunk(wt, dests, mt, sc, evac))

        def rope(dst, chunks=(0, 1)):
            # dst (in place): top = te*cos - to*sin ; bottom = te*sin + to*cos
            # ra = [cosT; cosT], rb = [-sinT; sinT]; swap = halves exchanged.
            for c in chunks:
                sl = slice(c * 1024, (c + 1) * 1024)
                # SWDGE (gpsimd) keeps this 1 queue -> 1 sem; a wide HWDGE
                # sbuf->sbuf DMA fans out over many queues and blows the
                # consumer's sync-wait slot budget.
                swp = pool.tile([128, 1024], F16, name="swp", tag="swp", bufs=2)
                nc.gpsimd.dma_start(swp[0:64, :], dst[64:128, sl])
                nc.gpsimd.dma_start(swp[64:128, :], dst[0:64, sl])
                u = pool.tile([128, 1024], F16, name="u", tag="sc", bufs=2)
                nc.vector.tensor_mul(u[:, :], dst[:, sl], ra[:, sl])
                v2 = pool.tile([128, 1024], F16, name="v2", tag="sc", bufs=2)
                nc.vector.tensor_mul(v2[:, :], swp[:, :], rb[:, sl])
                nc.vector.tensor_add(dst[:, sl], u[:, :], v2[:, :])

        def diag_unit(h, c):
            # ds_diag[32h, c-half] = per-position q.k (fp16 products,
            # fp32 psum accumulation via ones-matmul column sum)
            hp = 32 * h
            sl = slice(c * 1024, (c + 1) * 1024)
            pr = pool.tile([128, 1024], F16, name="pr", tag="pr", bufs=2)
            nc.vector.tensor_mul(pr[:, :], qh[h][:, sl], kh[h][:, sl])
            for cc in range(2):
                dps = pp.tile([128, 512], F32, name="dps", tag="mm", bufs=2)
                nc.tensor.matmul(dps[:, :], ones1[:, :],
                                 pr[:, cc * 512:(cc + 1) * 512],
                                 start=True, stop=True)
                o = (2 * c + cc) * 512
                nc.vector.tensor_copy(ds_diag[hp:hp + 1, o:o + 512],
                                      dps[hp:hp + 1, :])

        def expd_row(h):
            hp = 32 * h
            nc.scalar.activation(expd[hp:hp + 1, :], ds_diag[hp:hp + 1, :],
                                 AF.Exp)

        # accumulate the two half-sums of each sq into adjacent columns;
        # head_sum_tail adds them (a DVE reduce of the exp output would
        # cost ~2.3us/sq - reductions don't get the 16-bit 2x rate)
        sumh = [pool.tile([128, SB, 2], F32, name=f"sumh{h}")
                for h in range(HPC)]

        def sco_block(h, sq, half):
            # one 128x1024 score tile -> exp with fused key-axis row sum
            sps = pp.tile([128, 1024], F32, name="sps", tag="sco", bufs=2)
            for cc in range(2):
                o = half * 1024 + cc * 512
                nc.tensor.matmul(sps[:, cc * 512:(cc + 1) * 512],
                                 qh[h][:, sq * 128:(sq + 1) * 128],
                                 kh[h][:, o:o + 512],
                                 start=True, stop=True)
            ex = pool.tile([128, 1024], F16, name="ex", tag="ex", bufs=2)
            nc.scalar.activation(ex[:, :], sps[:, :], AF.Exp,
                                 accum_out=sumh[h][:, sq, half:half + 1])

        rsh = [pool.tile([128, SB], F32, name=f"rsh{h}")
               for h in range(HPC)]

        def hst_part(h, c0, c1):
            # sumexp columns [c0:c1): halves-add -> recip -> transpose ->
            # ds_sum row segment -> w segment.  Split so most of the
            # serial chain runs mid-blocks (exps for sq<c1 are already
            # done) and only a 4-column stub remains at the head boundary.
            n = c1 - c0
            hp = 32 * h
            nc.vector.tensor_add(sumf[h][:, c0:c1], sumh[h][:, c0:c1, 0],
                                 sumh[h][:, c0:c1, 1])
            nc.vector.reciprocal(rsh[h][:, c0:c1], sumf[h][:, c0:c1])
            tps = pp.tile([128, 512], F32, name="tps", tag="mm", bufs=2)
            nc.tensor.transpose(tps[0:n, 0:128], rsh[h][:, c0:c1],
                                ident[:, :])
            st = pool.tile([16, 128], F16, name="st", tag="st", bufs=2)
            nc.vector.tensor_copy(st[0:n, :], tps[0:n, 0:128])
            # sync (HWDGE) queue: tiny transfer, no fan-out concern; the
            # gpsimd queue would head-of-line block this behind rope swap
            # DMAs that wait on late projection fillers
            nc.sync.dma_start(
                ds_sum[hp:hp + 1, c0 * 128:c1 * 128], st[0:n, :])
            nc.vector.tensor_mul(w4[hp:hp + 1, c0 * 128:c1 * 128],
                                 expd[hp:hp + 1, c0 * 128:c1 * 128],
                                 ds_sum[hp:hp + 1, c0 * 128:c1 * 128])

        def head_weights(h, fill):
            # w = exp(diag) * recip(sumexp); attn = w (bcast) * v, into kh.
            # expd row precomputed; broadcast matmul output is consumed
            # directly from PSUM by the DVE multiply (no staging copy).
            hp = 32 * h
            for ck in range(SC):
                bps = pp.tile([128, 512], F32, name="bps", tag="op", bufs=2)
                nc.tensor.matmul(bps[:, :], ones1[hp:hp + 1, :],
                                 w4[hp:hp + 1, ck * 512:(ck + 1) * 512],
                                 start=True, stop=True,
                                 tile_position=(hp, 0))
                sl = slice(ck * 512, (ck + 1) * 512)
                nc.vector.tensor_mul(kh[h][:, sl], bps[:, :], vh[h][:, sl])
                fill(220)

        op_count = [0]
        tail_mode = [False]
        ytb_box = [None]

        # output pairs: the host sums all partials, so pair membership is
        # free.  (1,2) completes after head 2 - its output projection
        # feeds head 3's score blocks; (0,3) fills the tail.
        PAIRS = [(1, 2), (0, 3)]

        def oproj_group(p, sb, ncx, taper=False):
            # groups run ncx 0..3 for one sb-block; the four [128,512]
            # evacuations land in one staging tile and ship as a single
            # contiguous DMA (4KB rows, 1 sync-queue trigger per 4 groups).
            # taper: per-group DMAs for the last batches so the final
            # drain spreads across queues instead of one serial transfer.
            h0, h1 = PAIRS[p]
            wot = wot_box[0]
            op_count[0] += 1
            use_sco = tail_mode[0] and op_count[0] % 2
            if use_sco:
                # scores are done in the tail: borrow the sco banks for a
                # 4-deep psum rotation so the group stream is PE-paced
                big = pp.tile([128, 1024], F32, name="osps", tag="sco",
                              bufs=2)
                ps = big[:, 0:512]
            else:
                ps = pp.tile([128, 512], F32, name="ops", tag="op", bufs=2)
            for i, h in enumerate((h0, h1)):
                nc.tensor.matmul(
                    ps[:, :], kh[h][:, sb * 128:(sb + 1) * 128],
                    wot[:, h, ncx * 512:(ncx + 1) * 512],
                    start=(i == 0), stop=(i == 1))
            if taper:
                yt = pool.tile([128, 512], F16, name="ytt", tag="ytt",
                               bufs=4)
                dst = yt[:, :]
            else:
                if ncx == 0:
                    ytb_box[0] = pool.tile([128, S], F16, name="ytb",
                                           tag="yt", bufs=3)
                dst = ytb_box[0][:, ncx * 512:(ncx + 1) * 512]
            # DVE while ACT runs at the exp floor; alternate engines in
            # the PE-dense tail so neither one paces the group stream
            if use_sco:
                nc.scalar.activation(dst, ps[:, :], AF.Copy)
            else:
                nc.vector.tensor_copy(dst, ps[:, :])
            if taper:
                nc.sync.dma_start(
                    y[p, sb * 128:(sb + 1) * 128,
                      ncx * 512:(ncx + 1) * 512], dst)
            elif ncx == SC - 1:
                nc.sync.dma_start(
                    y[p, sb * 128:(sb + 1) * 128, :], ytb_box[0][:, :])

        # ================= filler queue =================
        # Units of (pe_cost_ns, emit_fn), drained between score blocks to
        # keep the PE streaming while ACT runs the exps.
        fillers = deque()
        fill_debt = [0.0]

        def fill(budget):
            fill_debt[0] += budget
            while fillers and fillers[0][0] <= fill_debt[0]:
                cost, fn = fillers.popleft()
                fn()
                fill_debt[0] -= cost

        def drain(dq):
            while dq:
                _, fn = dq.popleft()
                fn()

        # ================= emission =================
        # dense PE lead-in.  K proj sc0+sc1 run as 8 parallel
        # accumulation groups (all 8 psum banks) with matmuls kb-major,
        # so the PE streams in x's DMA arrival order.  Ropes are emitted
        # the moment their tensor completes so the DVE overlaps the
        # remaining PE chunks.
        lead_big = [pp.tile([128, 1024], F32, name="lps", tag="sco",
                            bufs=2) for _ in range(2)]
        lead_ps = ([t[:, 0:512] for t in lead_big]
                   + [t[:, 512:1024] for t in lead_big]
                   + [pp.tile([128, 512], F32, name="mmps", tag="mm",
                              bufs=2) for _ in range(2)]
                   + [pp.tile([128, 512], F32, name="olps", tag="op",
                              bufs=2) for _ in range(2)])
        for kb in range(KB):
            for g in range(8):
                mt, sc = g % HPC, g // HPC
                nc.tensor.matmul(
                    lead_ps[g], wkt[:, kb, mt * 128:(mt + 1) * 128],
                    xsb[:, kb, sc * 512:(sc + 1) * 512],
                    start=(kb == 0), stop=(kb == KB - 1))
        for g in range(8):
            mt, sc = g % HPC, g // HPC
            dst = kh[mt][:, sc * 512:(sc + 1) * 512]
            if g % 2:
                nc.scalar.activation(dst, lead_ps[g], AF.Copy)
            else:
                nc.vector.tensor_copy(dst, lead_ps[g])
        for sc in (2,):
            for mt in range(HPC):
                proj_chunk(wkt, kh, mt, sc)
        for mt in range(HPC):
            proj_chunk(wkt, kh, mt, SC - 1)
            rope(kh[mt])
        # wv reuses wk's slot, wo reuses wq's slot (tag bufs=2); the loads
        # self-delay on the WAR semaphore of the previous consumer.
        wvt = load_w(wv_r, KB)
        proj_chunk(wqt, qh, 0, 0)
        proj_chunk(wqt, qh, 0, 1)
        rope(qh[0], (0,))
        proj_chunk(wqt, qh, 0, 2)
        proj_chunk(wqt, qh, 0, 3)
        rope(qh[0], (1,))

        # Emission-time progress flags for the force-drain guards below
        # (all bookkeeping is emission-time python, fully deterministic).
        q_ready = [True] + [False] * (HPC - 1)
        v_done = [0] * HPC
        e_done = [False] * HPC

        def mark(fn, after):
            def wrapped():
                fn()
                after()
            return wrapped

        # queue: diag/expd h0, Q proj h1 (+rope/diag/expd), V proj h0/h1,
        # Q proj h2/h3, V proj h2/h3; pair-0 output proj appended later.
        for c in range(2):
            fillers.append((440, lambda c=c: diag_unit(0, c)))
        fillers.append(
            (0, mark(lambda: expd_row(0),
                     lambda: e_done.__setitem__(0, True))))
        wot_box = []

        def queue_qhead(hq):
            for sc in range(SC):
                fillers.append(proj_unit(wqt, qh, hq, sc, evac="dve"))
            fillers.append((0, lambda hq=hq: rope(qh[hq], (0,))))
            fillers.append(
                (0, mark(lambda hq=hq: rope(qh[hq], (1,)),
                         lambda hq=hq: q_ready.__setitem__(hq, True))))
            for c in range(2):
                fillers.append((440, lambda hq=hq, c=c: diag_unit(hq, c)))
            fillers.append(
                (0, mark(lambda hq=hq: expd_row(hq),
                         lambda hq=hq: e_done.__setitem__(hq, True))))

        def queue_vhead(mt):
            for sc in range(SC):
                fillers.append(
                    (3460, mark(
                        lambda mt=mt, sc=sc:
                        proj_chunk(wvt, vh, mt, sc, evac="dve"),
                        lambda mt=mt:
                        v_done.__setitem__(mt, v_done[mt] + 1))))

        queue_qhead(1)
        queue_vhead(0)
        queue_vhead(1)
        queue_qhead(2)
        queue_vhead(2)
        queue_qhead(3)
        # wo load directly after the last wqt-consuming unit
        fillers.append((0, lambda: wot_box.append(load_w(wo_r, HPC))))
        queue_vhead(3)

        FILL_A = 840    # ns of filler per score block: ACT exp+accum
        FILL_B = 840    # paces a block at ~1.3us, 432ns are score MMs;
        # PE period must stay >= the ACT period or the PE micro-waits
        # on score PSUM reuse every block and HAM drops to half clock

        reserve = deque()

        def rfill(budget):
            fill_debt[0] += budget
            while reserve and reserve[0][0] <= fill_debt[0]:
                cost, fn = reserve.popleft()
                fn()
                fill_debt[0] -= cost

        def force(cond):
            # pop fillers (in order) until an emission-order precondition
            # holds; keeps DVE/PE FIFO deps acyclic regardless of budgets
            while fillers and not cond():
                _, fn = fillers.popleft()
                fn()
            assert cond()

        def emit_head_blocks(h, per_block):
            force(lambda: q_ready[h])
            for sq in range(SB):
                for half in range(2):
                    sco_block(h, sq, half)
                    fill(per_block)
                if sq == 12:
                    force(lambda: e_done[h])
                    hst_part(h, 0, 12)

        def pair_tail(h, f):
            # per-head: finish the last sumexp columns, then w + attn,
            # right after the head's last score block; ck3 goes last so
            # its chain hides under the ck0-2 broadcast+attn work
            f(4000)
            hst_part(h, 12, SB)
            force(lambda: v_done[h] >= SC)
            f(2500)
            head_weights(h, f)

        emit_head_blocks(0, FILL_A)
        pair_tail(0, fill)
        emit_head_blocks(1, FILL_A)
        pair_tail(1, fill)
        emit_head_blocks(2, FILL_B)
        pair_tail(2, fill)

        # pair (1,2) output projection becomes available; it feeds head
        # 3's blocks (8 units reserved for the head-3 transform window)
        oq = [(s_, n_) for s_ in range(SB) for n_ in range(SC)]
        for i, (s_, n_) in enumerate(oq):
            unit = (432, lambda s_=s_, n_=n_: oproj_group(0, s_, n_))
            (reserve if i >= len(oq) - 8 else fillers).append(unit)

        emit_head_blocks(3, FILL_B)
        drain(fillers)
        pair_tail(3, rfill)
        tail_mode[0] = True
        drain(reserve)

        # ---- tail: pair (0,3) output projection ----
        for sb in range(SB):
            for ncx in range(SC):
                oproj_group(1, sb, ncx)

    nc.compile()
    return nc


def _get_nc():
    if "nc" not in _CACHE:
        _CACHE["nc"] = _build_nc()
    return _CACHE["nc"]


_PERM = np.concatenate([np.arange(0, DH, 2), np.arange(1, DH, 2)])


def _host_inputs(x, rope_cos, rope_sin, Wq, Wk, Wv, Wo):
    """Build the 8 per-core input maps."""
    f16 = np.float16
    cosT = np.ascontiguousarray(np.asarray(rope_cos, np.float32)[0, :, 0, :].T)
    sinT = np.ascontiguousarray(np.asarray(rope_sin, np.float32)[0, :, 0, :].T)
    ra = np.concatenate([cosT, cosT], 0).astype(f16)
    rb = np.concatenate([-sinT, sinT], 0).astype(f16)

    Wq = np.asarray(Wq, np.float32)
    Wk = np.asarray(Wk, np.float32)
    Wv = np.asarray(Wv, np.float32)
    Wo = np.asarray(Wo, np.float32)
    x = np.asarray(x, np.float32)

    xTb = [np.ascontiguousarray(x[b].T).astype(f16) for b in range(B)]
    scale = DH ** -0.5

    in_maps = []
    for core in range(NCORES):
        b, g = divmod(core, HPC)
        hs = g * HPC
        rows = np.concatenate(
            [h * DH + _PERM for h in range(hs, hs + HPC)])      # deinterleave
        rows_v = np.arange(hs * DH, (hs + HPC) * DH)
        in_maps.append({
            "xT": xTb[b],
            "wq": np.ascontiguousarray((Wq[rows] * scale).T).astype(f16),
            "wk": np.ascontiguousarray(Wk[rows].T).astype(f16),
            "wv": np.ascontiguousarray(Wv[rows_v].T).astype(f16),
            "wo": np.ascontiguousarray(Wo[:, rows_v].T).astype(f16),
            "ropeA": ra,
            "ropeB": rb,
        })
    return in_maps


def kernel(x, rope_cos, rope_sin, Wq, Wk, Wv, Wo, _trace=False, _trace_cores=None):
    from concourse.bass_utils import run_bass_kernel_spmd

    nc = _get_nc()
    in_maps = _host_inputs(x, rope_cos, rope_sin, Wq, Wk, Wv, Wo)
    res = run_bass_kernel_spmd(nc, in_maps, list(range(NCORES)),
                               trace=_trace, trace_cores=_trace_cores)
    _CACHE["last_result"] = res

    out = np.zeros((B, S, D), np.float32)
    for core in range(NCORES):
        b = core // HPC
        out[b] += res.results[core]["y"].astype(np.float32).sum(axis=0)
    return out
